# revision 16
# baseline (speedup 1.0000x reference)
"""Trainium2 Bass kernel for nn_AttentionBlock (GroupNorm + linear attention + proj + residual).

Full shapes: x [4, 256, 32, 32, 32] fp32, N = 32768 spatial positions.

Reference computation:
  norm = GroupNorm(4 groups)(x);  qkv = qkv_weight @ norm (1x1x1 conv)
  k = softmax(k, axis=spatial);  sim[h] = k[h] @ v[h].T  (hd x hd)
  out[h] = sim[h].T @ q[h];  out = out_weight @ out + out_bias + x

Sharding (8 cores): core c -> batch b = c//2, spatial half h2 = c%2.
Each core:
  - streams its x[b][:, half] (16 MB fp32), computing partial GN stats while
    caching x as fp16 in SBUF (fp16 carries the same 10-bit mantissa as the
    tf32 matmul path, so matmul precision is ~unchanged vs fp32r)
  - AllReduce (pair) of per-channel sum/sumsq -> fold GN into qkv weights
  - phase A: exp(kT)/vT via fp16 matmuls (n on partitions), sim+denominator
    accumulated in PSUM over all local n; AllReduce (pair) of sim partials
  - fold: att weights W2 = a_c * (q_weight.T @ sim_blockdiag); ab = sim.T@qbias
  - phase B: att = W2.T@x + ab, proj, +bias +residual for its n-half

Algebraic tricks (validated vs reference):
  - GN fold: qkv(norm(x)) = (W * a_c) @ x + W @ b_c; a,b from group stats
  - k bias dropped entirely (softmax shift invariance)
  - softmax denominator = extra ones-column in the sim matmul rhs
  - v bias folded post-hoc: sim_norm = sim_raw/den + vbias (rank-1 via denom)
  - sim folded into q weights (skips materializing q entirely)
  - residual added via identity-matmul into the proj PSUM accumulation
"""
import numpy as np

import concourse.bass as bass
import concourse.bacc as bacc
import concourse.mybir as mybir
import concourse.tile as tile
from concourse import bass_utils

N_CORES = 8
B, C, Dd, Hh, Ww = 4, 256, 32, 32, 32
N = Dd * Hh * Ww           # 32768
NH = N // 2                # 16384 (per-core spatial half)
G = 4                      # groupnorm groups
EPS = 1e-5
f32 = mybir.dt.float32
f16 = mybir.dt.float16
AF = mybir.ActivationFunctionType
ALU = mybir.AluOpType
AX = mybir.AxisListType

REPLICA_GROUPS = [[0, 1], [2, 3], [4, 5], [6, 7]]


def build(nh=NH):
    """Build + compile the SPMD program. nh parameterized for fast sim tests."""
    stats_ch = min(2048, nh)
    n_stats_ch = nh // stats_ch
    n_pair = nh // 256         # phase A processes 2x128-col sub-chunks per iter
    n_blk = nh // 512          # phase B 512-col blocks
    inv_n = 1.0 / (64.0 * 2 * nh)   # group stats count: 64 ch x full N (=2*nh)

    nc = bacc.Bacc("TRN2", target_bir_lowering=False, debug=False,
                   num_devices=N_CORES)

    xh_d = nc.dram_tensor("xh", [2, 128, nh], f32, kind="ExternalInput")
    kvw_d = nc.dram_tensor("kvw", [2, 128, 512], f32, kind="ExternalInput")
    qw_d = nc.dram_tensor("qw", [2, 128, 256], f32, kind="ExternalInput")
    qw2_d = nc.dram_tensor("qw2", [2, 128, 256], f32, kind="ExternalInput")
    ow_d = nc.dram_tensor("ow", [2, 128, 256], f32, kind="ExternalInput")
    gnw_d = nc.dram_tensor("gnw", [2, 128, 1], f32, kind="ExternalInput")
    gnb_d = nc.dram_tensor("gnb", [2, 128, 1], f32, kind="ExternalInput")
    ind_d = nc.dram_tensor("ind", [2, 128, 4], f32, kind="ExternalInput")
    indT_d = nc.dram_tensor("indT", [2, 4, 128], f32, kind="ExternalInput")
    mask_d = nc.dram_tensor("mask", [128, 128], f32, kind="ExternalInput")
    eye_d = nc.dram_tensor("eye", [128, 128], f16, kind="ExternalInput")
    ob_d = nc.dram_tensor("ob", [2, 128, 1], f32, kind="ExternalInput")
    out_d = nc.dram_tensor("out", [2, 128, nh], f32, kind="ExternalOutput")

    with tile.TileContext(nc) as tc:
        with tc.tile_pool(name="const", bufs=1) as cp, \
             tc.tile_pool(name="dram", bufs=1, space="DRAM") as dp:
            # ---- persistent SBUF tiles ----
            xc = [cp.tile([128, nh], f16, name=f"xc{t}", tag=f"xc{t}") for t in range(2)]
            kvw = [cp.tile([128, 512], f32, name=f"kvw{t}", tag=f"kvw{t}") for t in range(2)]
            kvws = [cp.tile([128, 512], f16, name=f"kvws{t}", tag=f"kvws{t}") for t in range(2)]
            qw = [cp.tile([128, 256], f32, name=f"qw{t}", tag=f"qw{t}") for t in range(2)]
            qw2 = [cp.tile([128, 256], f32, name=f"qw2{t}", tag=f"qw2{t}") for t in range(2)]
            owf = [cp.tile([128, 256], f32, name=f"owf{t}", tag=f"owf{t}") for t in range(2)]
            W3 = [cp.tile([128, 256], f16, name=f"W3{t}", tag=f"W3{t}") for t in range(2)]
            ab_col = [cp.tile([128, 1], f32, name=f"abc{t}", tag=f"abc{t}") for t in range(2)]
            ob2 = [cp.tile([128, 1], f32, name=f"ob2{t}", tag=f"ob2{t}") for t in range(2)]
            gnw = [cp.tile([128, 1], f32, name=f"gnw{t}", tag=f"gnw{t}") for t in range(2)]
            gnb = [cp.tile([128, 1], f32, name=f"gnb{t}", tag=f"gnb{t}") for t in range(2)]
            ind = [cp.tile([128, 4], f32, name=f"ind{t}", tag=f"ind{t}") for t in range(2)]
            indT = [cp.tile([4, 128], f32, name=f"indT{t}", tag=f"indT{t}") for t in range(2)]
            mask = cp.tile([128, 128], f32, name="mask", tag="mask")
            eye16 = cp.tile([128, 128], f16, name="eye16", tag="eye16")
            ob = [cp.tile([128, 1], f32, name=f"ob{t}", tag=f"ob{t}") for t in range(2)]
            ones_row = cp.tile([1, 128], f32, name="ones_row", tag="ones_row")
            a_sb = [cp.tile([128, 1], f32, name=f"a{t}", tag=f"a{t}") for t in range(2)]
            b_sb = [cp.tile([128, 1], f32, name=f"b{t}", tag=f"b{t}") for t in range(2)]
            qb_sb = [cp.tile([128, 1], f32, name=f"qb{t}", tag=f"qb{t}") for t in range(2)]
            vb_sb = cp.tile([1, 256], f32, name="vb", tag="vb")
            vbb_sb = [cp.tile([128, 128], f32, name=f"vbb{t}", tag=f"vbb{t}") for t in range(2)]
            simbd = [cp.tile([128, 128], f32, name=f"simbd{t}", tag=f"simbd{t}") for t in range(2)]

            for t in range(2):
                nc.gpsimd.dma_start(kvw[t][:], kvw_d.ap()[t])
                nc.gpsimd.dma_start(qw[t][:], qw_d.ap()[t])
                nc.gpsimd.dma_start(qw2[t][:], qw2_d.ap()[t])
                nc.gpsimd.dma_start(gnw[t][:], gnw_d.ap()[t])
                nc.gpsimd.dma_start(gnb[t][:], gnb_d.ap()[t])
                nc.gpsimd.dma_start(ind[t][:], ind_d.ap()[t])
                nc.gpsimd.dma_start(indT[t][:], indT_d.ap()[t])
                nc.gpsimd.dma_start(ob[t][:], ob_d.ap()[t])
            nc.gpsimd.dma_start(mask[:], mask_d.ap())
            nc.gpsimd.dma_start(eye16[:], eye_d.ap())
            nc.vector.memset(ones_row[:], 1.0)
            wu_in = dp.tile([1, 16], f32, name="wu_in", tag="wu_in")
            wu_out = dp.tile([1, 16], f32, name="wu_out", tag="wu_out")
            wu_sb = cp.tile([1, 16], f32, name="wu_sb", tag="wu_sb")
            nc.vector.memset(wu_sb[:], 0.0)
            nc.gpsimd.dma_start(wu_in[:], wu_sb[:])
            nc.gpsimd.collective_compute(
                "AllReduce", ALU.add, replica_groups=REPLICA_GROUPS,
                ins=[wu_in[:].opt()], outs=[wu_out[:].opt()])

            # ---- x load + cast-to-fp16 cache + partial stats ----
            with tc.tile_pool(name="sp", bufs=1) as sp, \
                 tc.tile_pool(name="spp", bufs=1, space="PSUM") as spp:
                for t in range(2):
                    nc.gpsimd.dma_start(owf[t][:], ow_d.ap()[t])

                scol_s = [sp.tile([128, n_stats_ch], f32, name=f"scs{t}", tag=f"scs{t}") for t in range(2)]
                scol_q = [sp.tile([128, n_stats_ch], f32, name=f"scq{t}", tag=f"scq{t}") for t in range(2)]
                stat2 = [sp.tile([128, 2], f32, name=f"st{t}", tag=f"st{t}") for t in range(2)]
                stat2r = [sp.tile([128, 2], f32, name=f"str{t}", tag=f"str{t}") for t in range(2)]

                for t in range(2):
                    for i in range(n_stats_ch):
                        sl = slice(i * stats_ch, (i + 1) * stats_ch)
                        xf = sp.tile([128, stats_ch], f32, name="xf", tag="xf", bufs=6)
                        nc.sync.dma_start(xf[:], xh_d.ap()[t, :, sl])
                        scr = sp.tile([128, stats_ch], f16, name="scr", tag="scr", bufs=2)
                        nc.scalar.activation(scr[:], xf[:], AF.Square,
                                             accum_out=scol_q[t][:, i:i + 1])
                        # cast to fp16 cache + per-channel sum in one DVE pass
                        nc.vector.tensor_scalar(xc[t][:, sl], xf[:], 1.0, None,
                                                op0=ALU.mult, op1=ALU.add,
                                                accum_out=scol_s[t][:, i:i + 1])
                    nc.vector.reduce_sum(stat2[t][:, 0:1], scol_s[t][:], axis=AX.X)
                    nc.vector.reduce_sum(stat2[t][:, 1:2], scol_q[t][:], axis=AX.X)

                # pair AllReduce of per-channel partial stats
                st_in = dp.tile([2, 128, 2], f32, name="st_in", tag="st_in")
                st_out = dp.tile([2, 128, 2], f32, name="st_out", tag="st_out")
                for t in range(2):
                    nc.sync.dma_start(st_in[t], stat2[t][:])
                nc.gpsimd.collective_compute(
                    "AllReduce", ALU.add, replica_groups=REPLICA_GROUPS,
                    ins=[st_in[:].opt()], outs=[st_out[:].opt()])
                for t in range(2):
                    nc.sync.dma_start(stat2r[t][:], st_out[t])

                # group stats: [4,2] = indicator.T @ (sum|sumsq)
                gps = spp.tile([4, 2], f32, name="gps", tag="gps")
                for t in range(2):
                    nc.tensor.matmul(gps[:], ind[t][:], stat2r[t][:],
                                     start=(t == 0), stop=(t == 1))
                eps4 = sp.tile([4, 1], f32, name="eps4", tag="eps4")
                nc.vector.memset(eps4[:], EPS)
                dml = sp.tile([1, 1], f32, name="dml", tag="dml")
                nc.vector.memset(dml[:], 1.0)
                nc.scalar.activation(dml[:], dml[:], AF.Ln)
                ms = sp.tile([4, 2], f32, name="ms", tag="ms")
                msq = sp.tile([4, 1], f32, name="msq", tag="msq")
                var = sp.tile([4, 1], f32, name="var", tag="var")
                lnv = sp.tile([4, 1], f32, name="lnv", tag="lnv")
                rstd = sp.tile([4, 1], f32, name="rstd", tag="rstd")
                rm = sp.tile([4, 2], f32, name="rm", tag="rm")
                nc.vector.tensor_scalar_mul(ms[:], gps[:], inv_n)
                nc.vector.tensor_mul(msq[:], ms[:, 0:1], ms[:, 0:1])
                nc.vector.tensor_sub(var[:], ms[:, 1:2], msq[:])
                nc.scalar.activation(lnv[:], var[:], AF.Ln, bias=eps4[:])
                nc.scalar.activation(rstd[:], lnv[:], AF.Exp, scale=-0.5)
                nc.vector.tensor_copy(rm[:, 0:1], rstd[:])
                nc.vector.tensor_copy(rm[:, 1:2], ms[:, 0:1])

                # broadcast to per-channel: chan[t] = indT.T @ (rstd|mean)
                ma = [sp.tile([128, 1], f32, name=f"ma{t}", tag=f"ma{t}") for t in range(2)]
                for t in range(2):
                    chan = spp.tile([128, 2], f32, name=f"chan{t}", tag=f"chan{t}")
                    nc.tensor.matmul(chan[:], indT[t][:], rm[:])
                    nc.vector.tensor_mul(a_sb[t][:], chan[:, 0:1], gnw[t][:])
                    nc.vector.tensor_mul(ma[t][:], chan[:, 1:2], a_sb[t][:])
                    nc.vector.tensor_sub(b_sb[t][:], gnb[t][:], ma[t][:])
                    # fold GN scale into kv weights (fp16 rounded on write)
                    nc.vector.tensor_scalar_mul(kvws[t][:], kvw[t][:], a_sb[t][:])

                # q bias: qb[dt] = qwT.T @ b_fold   (unscaled qw)
                for dt in range(2):
                    qb_ps = spp.tile([128, 1], f32, name=f"qbp{dt}", tag=f"qbp{dt}")
                    for t in range(2):
                        nc.tensor.matmul(qb_ps[:], qw[t][:, dt * 128:(dt + 1) * 128],
                                         b_sb[t][:], start=(t == 0), stop=(t == 1))
                    nc.vector.tensor_copy(qb_sb[dt][:], qb_ps[:])
                # v bias row: vb = b_fold.T @ vwT
                vb_ps = spp.tile([1, 256], f32, name="vbp", tag="vbp")
                for t in range(2):
                    nc.tensor.matmul(vb_ps[:], b_sb[t][:], kvw[t][:, 256:512],
                                     start=(t == 0), stop=(t == 1))
                nc.vector.tensor_copy(vb_sb[:], vb_ps[:])
                # broadcast vbias rows across partitions (rank-1 with ones)
                for dt in range(2):
                    vbb_ps = spp.tile([128, 128], f32, name=f"vbbp{dt}", tag=f"vbbp{dt}")
                    nc.tensor.matmul(vbb_ps[:], ones_row[:],
                                     vb_sb[:, dt * 128:(dt + 1) * 128])
                    nc.vector.tensor_copy(vbb_sb[dt][:], vbb_ps[:])

            # ---- phase A: exp(kT), vT, sim accumulation (fp16 matmuls) ----
            with tc.tile_pool(name="pa", bufs=1) as pa, \
                 tc.tile_pool(name="pap", bufs=1, space="PSUM") as pap:
                wu2_in = dp.tile([1, 16], f32, name="wu2_in", tag="wu2_in")
                wu2_out = dp.tile([1, 16], f32, name="wu2_out", tag="wu2_out")
                nc.gpsimd.dma_start(wu2_in[:], wu_sb[:])
                nc.gpsimd.collective_compute(
                    "AllReduce", ALU.add, replica_groups=REPLICA_GROUPS,
                    ins=[wu2_in[:].opt()], outs=[wu2_out[:].opt()])
                sim_ps = [pap.tile([128, 129], f32, name=f"sim{dt}", tag=f"sim{dt}") for dt in range(2)]
                for p in range(n_pair):
                    kv_ps = pap.tile([128, 1024], f32, name="kv", tag="kv", bufs=3)
                    for s2 in range(2):
                        s = 2 * p + s2
                        sl = slice(s * 128, (s + 1) * 128)
                        nc.tensor.matmul(kv_ps[:, s2 * 512:(s2 + 1) * 512],
                                         xc[0][:, sl], kvws[0][:],
                                         start=True, stop=False)
                        nc.tensor.matmul(kv_ps[:, s2 * 512:(s2 + 1) * 512],
                                         xc[1][:, sl], kvws[1][:],
                                         start=False, stop=True)
                    ek = pa.tile([128, 512], f16, name="ek", tag="ek", bufs=3)
                    # k cols of the two sub-chunks: [p, (s2, 0:256 of 512)]
                    kv_k = kv_ps[:].rearrange("p (s d) -> p s d", s=2)[:, :, 0:256]
                    ek2 = ek[:].rearrange("p (s d) -> p s d", s=2)
                    nc.scalar.activation(ek2, kv_k, AF.Exp)
                    vt = pa.tile([128, 516], f16, name="vt", tag="vt", bufs=3)
                    # v cols -> [s2][dt] blocks of 128, each followed by a ones col
                    kv_v = kv_ps[:].rearrange("p (s d c) -> p s d c", s=2, d=4)[:, :, 2:4, :]
                    vt4 = vt[:].rearrange("p (s d c) -> p s d c", s=2, d=2)
                    nc.vector.tensor_copy(vt4[:, :, :, 0:128], kv_v)
                    nc.vector.memset(vt4[:, :, :, 128:129], 1.0)
                    first, last = (p == 0), (p == n_pair - 1)
                    for s2 in range(2):
                        for dt in range(2):
                            nc.tensor.matmul(
                                sim_ps[dt][:],
                                ek[:, s2 * 256 + dt * 128: s2 * 256 + (dt + 1) * 128],
                                vt[:, s2 * 258 + dt * 129: s2 * 258 + (dt + 1) * 129],
                                start=(first and s2 == 0), stop=(last and s2 == 1))

                # pair AllReduce of sim partials (+denominator column)
                sim_sb = [pa.tile([128, 129], f32, name=f"simsb{dt}", tag=f"simsb{dt}") for dt in range(2)]
                simr = [pa.tile([128, 129], f32, name=f"simr{dt}", tag=f"simr{dt}") for dt in range(2)]
                si_in = dp.tile([2, 128, 129], f32, name="si_in", tag="si_in")
                si_out = dp.tile([2, 128, 129], f32, name="si_out", tag="si_out")
                for dt in range(2):
                    nc.vector.tensor_copy(sim_sb[dt][:], sim_ps[dt][:])
                    nc.sync.dma_start(si_in[dt], sim_sb[dt][:])
                nc.gpsimd.collective_compute(
                    "AllReduce", ALU.add, replica_groups=REPLICA_GROUPS,
                    ins=[si_in[:].opt()], outs=[si_out[:].opt()])
                for dt in range(2):
                    nc.sync.dma_start(simr[dt][:], si_out[dt])

                # normalize + vbias + block-diag mask
                for dt in range(2):
                    recip = pa.tile([128, 1], f32, name=f"rec{dt}", tag=f"rec{dt}")
                    simn = pa.tile([128, 128], f32, name=f"simn{dt}", tag=f"simn{dt}")
                    nc.vector.reciprocal(recip[:], simr[dt][:, 128:129])
                    nc.vector.scalar_tensor_tensor(
                        simn[:], simr[dt][:, 0:128], recip[:], vbb_sb[dt][:],
                        op0=ALU.mult, op1=ALU.add)
                    nc.vector.tensor_mul(simbd[dt][:], simn[:], mask[:])

            # ---- fold sim+proj into one matrix: out = W3.T@x + ob2 + x ----
            # W2rawT[et] = simbd[et].T @ qw2[et]   ([e, c])
            # W3[ct] = a_c * sum_et W2rawT[et][:, ct].T @ owT[et]   ([c, o])
            # ob2[ot] = sum_et owT[et][:, ot].T @ (simbd[et].T @ qb[et]) + out_bias
            with tc.tile_pool(name="pwsb", bufs=1) as pwsb, \
                 tc.tile_pool(name="pw", bufs=1, space="PSUM") as pw:
                w2rt = [pwsb.tile([128, 256], f32, name=f"w2rt{et}", tag=f"w2rt{et}")
                        for et in range(2)]
                for et in range(2):
                    w2_ps = pw.tile([128, 256], f32, name=f"w2p{et}", tag=f"w2p{et}")
                    nc.tensor.matmul(w2_ps[:], simbd[et][:], qw2[et][:])
                    nc.vector.tensor_copy(w2rt[et][:], w2_ps[:])
                for ct in range(2):
                    w3_ps = pw.tile([128, 256], f32, name=f"w3p{ct}", tag=f"w3p{ct}")
                    for et in range(2):
                        nc.tensor.matmul(w3_ps[:], w2rt[et][:, ct * 128:(ct + 1) * 128],
                                         owf[et][:], start=(et == 0), stop=(et == 1))
                    nc.vector.tensor_scalar_mul(W3[ct][:], w3_ps[:], a_sb[ct][:])
                    nc.vector.tensor_add(W3[ct][:, ct * 128:(ct + 1) * 128],
                                         W3[ct][:, ct * 128:(ct + 1) * 128],
                                         eye16[:])
                for et in range(2):
                    ab_ps = pw.tile([128, 1], f32, name=f"abp{et}", tag=f"abp{et}")
                    nc.tensor.matmul(ab_ps[:], simbd[et][:], qb_sb[et][:])
                    nc.vector.tensor_copy(ab_col[et][:], ab_ps[:])
                for ot in range(2):
                    ob2_ps = pw.tile([128, 1], f32, name=f"ob2p{ot}", tag=f"ob2p{ot}")
                    for et in range(2):
                        nc.tensor.matmul(ob2_ps[:], owf[et][:, ot * 128:(ot + 1) * 128],
                                         ab_col[et][:], start=(et == 0), stop=(et == 1))
                    nc.vector.tensor_add(ob2[ot][:], ob2_ps[:], ob[ot][:])

            # ---- phase B: att = W2.T@x + ab, proj, bias+residual ----
            with tc.tile_pool(name="pb", bufs=1) as pb, \
                 tc.tile_pool(name="pbp", bufs=4, space="PSUM") as pbp:
                for blk in range(n_blk):
                    sl = slice(blk * 512, (blk + 1) * 512)
                    for ot in range(2):
                        pr_ps = pbp.tile([128, 512], f32, name=f"mm{ot}", tag=f"mm{ot}")
                        nc.tensor.matmul(pr_ps[:], W3[0][:, ot * 128:(ot + 1) * 128],
                                         xc[0][:, sl], start=True, stop=False)
                        nc.tensor.matmul(pr_ps[:], W3[1][:, ot * 128:(ot + 1) * 128],
                                         xc[1][:, sl], start=False, stop=True)
                        o = pb.tile([128, 512], f32, name=f"os{ot}", tag=f"os{ot}", bufs=3)
                        if ot == 0:
                            nc.scalar.activation(o[:], pr_ps[:], AF.Identity,
                                                 bias=ob2[ot][:])
                        else:
                            nc.vector.tensor_scalar_add(o[:], pr_ps[:], ob2[ot][:])
                        nc.sync.dma_start(out_d.ap()[ot, :, sl], o[:])

    nc.compile()
    return nc


_NC = None


def _get_nc():
    global _NC
    if _NC is None:
        _NC = build()
    return _NC


def make_in_maps(x, gn_weight, gn_bias, qkv_weight, out_weight, out_bias, nh=NH):
    x = np.ascontiguousarray(x, dtype=np.float32)
    qkv_weight = np.asarray(qkv_weight, dtype=np.float32)
    out_weight = np.asarray(out_weight, dtype=np.float32)
    n = 2 * nh

    kvwT = np.ascontiguousarray(
        np.concatenate([qkv_weight[C:2 * C], qkv_weight[2 * C:3 * C]], axis=0).T
    ).reshape(2, 128, 512)
    qwT = np.ascontiguousarray(qkv_weight[0:C].T).reshape(2, 128, 256)
    qw2 = np.ascontiguousarray(qkv_weight[0:C]).reshape(2, 128, 256)
    owT = np.ascontiguousarray(out_weight.T).reshape(2, 128, 256)
    gnw = np.ascontiguousarray(gn_weight, dtype=np.float32).reshape(2, 128, 1)
    gnb = np.ascontiguousarray(gn_bias, dtype=np.float32).reshape(2, 128, 1)
    obp = np.ascontiguousarray(out_bias, dtype=np.float32).reshape(2, 128, 1)
    ind = np.zeros((C, G), np.float32)
    ind[np.arange(C), np.arange(C) // 64] = 1.0
    indT = np.ascontiguousarray(ind.T)
    ind = ind.reshape(2, 128, 4)
    indT = np.stack([indT[:, 0:128], indT[:, 128:256]]).copy()  # [2,4,128]
    mask = np.zeros((128, 128), np.float32)
    for h in range(4):
        mask[h * 32:(h + 1) * 32, h * 32:(h + 1) * 32] = 1.0
    eye = np.eye(128, dtype=np.float16)

    shared = {"kvw": kvwT, "qw": qwT, "qw2": qw2, "ow": owT, "gnw": gnw,
              "gnb": gnb, "ind": ind, "indT": indT, "mask": mask, "eye": eye,
              "ob": obp}
    in_maps = []
    for c in range(N_CORES):
        b, h2 = c // 2, c % 2
        xb = x[b].reshape(C, n)
        xh = np.ascontiguousarray(xb[:, h2 * nh:(h2 + 1) * nh]).reshape(2, 128, nh)
        in_maps.append({"xh": xh, **shared})
    return in_maps


def assemble(results, nh=NH):
    n = 2 * nh
    out = np.empty((B, C, n), np.float32)
    for c in range(N_CORES):
        b, h2 = c // 2, c % 2
        out[b][:, h2 * nh:(h2 + 1) * nh] = results[c]["out"].reshape(C, nh)
    return out


def kernel(x, gn_weight, gn_bias, qkv_weight, out_weight, out_bias):
    nc = _get_nc()
    in_maps = make_in_maps(x, gn_weight, gn_bias, qkv_weight, out_weight, out_bias)
    res = bass_utils.run_bass_kernel_spmd(nc, in_maps, core_ids=list(range(N_CORES)))
    return assemble(res.results).reshape(B, C, Dd, Hh, Ww)
